# revision 45
# baseline (speedup 1.0000x reference)
"""2-layer 8-head GAT forward, distributed over 8 Trainium2 NeuronCores.

Strategy (graph data parallelism, per sharding hint):
  - Edges sorted by destination; dst nodes blocked by 128; core c owns the 10
    blocks covering nodes [c*1280, (c+1)*1280). Index preprocessing host-side.
  - Per layer one DRAM table (bf16), row for node n (block-major layout):
      [ h(n) 256 bf16 | p=exp(s) 8 f32 | q=exp(0.2 s) 8 f32 | pad ]  (768 B)
    h = x @ W with the attention vectors folded in (W_ext = [W|W@As|W@Ad]).
    Layer-1 table: built redundantly on every core from the full x (no
    collectives). Layer-2 table: each core builds the rows for its own nodes
    inside the layer-1 epilogue (PE transpose of z + matmul); pairs of
    128-row blocks are AllGather'd straight into the table (group-major
    layout, see AGB), overlapped with the remaining layer-1 edge work.
  - The dst-side attention terms exp(d), exp(0.2 d) stay SBUF-resident on the
    owning core (dst nodes of a core's blocks are its own nodes) — no z
    exchange and no redundant layer-2 table rebuild.
  - Edge phase per dst block:
      * batched dma_gather of fat rows by src id (1024-idx chunks, 4 queues)
      * per 128-edge tile, host-supplied selection matrices (fp8 0/1):
          sld[d,e] expands [exp(d)|exp(0.2d)] to edges via a PE matmul (N=16)
          slt[e,d] accumulates rhs = [ex*h | ex] into PSUM (N=264)
        with ex = exp(leaky(s+d)) = max(exp(s)exp(d), exp(.2s)exp(.2d)).
      * epilogue divides by the summed ex, adds bias (+ ELU after layer 1).
Output: each core writes its 1280 dst rows f32; host concatenates and trims.
"""

import os
import sys

for _p in ("/opt/trn_rl_repo", "/root/.axon_site/_ro/trn_rl_repo"):
    if os.path.isdir(_p) and _p not in sys.path:
        sys.path.append(_p)

import numpy as np

from concourse import bass, bacc, mybir
import concourse.tile as tile
from concourse.masks import make_identity
from concourse.bass_utils import run_bass_kernel_spmd

F32 = mybir.dt.float32
BF16 = mybir.dt.bfloat16
FP8 = mybir.dt.float8e4
I16 = mybir.dt.int16
AF = mybir.ActivationFunctionType
OP = mybir.AluOpType
P = 128
AGB = [(0, 2), (2, 4), (4, 6), (6, 8), (8, 10)]  # AllGather block groups


class Cfg:
    def __init__(self, n_nodes=10000, n_edges=320000, hid=256, heads=8, n_cores=8):
        self.N = n_nodes
        self.E = n_edges
        self.HID = hid
        self.H = heads
        self.C = hid // heads
        self.NC = n_cores
        self.NP = -(-n_nodes // (P * n_cores)) * (P * n_cores)
        self.NT = self.NP // P
        self.BPC = self.NT // n_cores
        self.NW = hid + 2 * heads        # table matmul width (h | s | d)
        self.NW2 = hid + heads           # edge matmul rhs width (ex*h | ex)
        self.FAT = hid + 128             # fat row elems (bf16): h | p | q | pad
        self.PQ0 = hid                   # bf16 slot where p (f32 x8) starts
        self.PQ1 = hid + 32              # end of q (f32 x8)
        assert self.FAT * 2 % 256 == 0


# --------------------------------------------------------------------------
# Host preprocessing (indices / selection masks / weight folding)
# --------------------------------------------------------------------------
def _group_maps(cfg):
    """Per-block-index maps for the group-major table layout (AGB)."""
    GBASE = np.zeros(cfg.BPC, np.int64)
    GSIZE = np.zeros(cfg.BPC, np.int64)
    GOFF = np.zeros(cfg.BPC, np.int64)
    for b0, b1 in AGB:
        GBASE[b0:b1] = b0 * cfg.NC * P
        GSIZE[b0:b1] = (b1 - b0) * P
        GOFF[b0:b1] = np.arange(b1 - b0)
    return GBASE, GSIZE, GOFF


def _wrap16(idx, T):
    w = idx.reshape(-1, 16).T.astype(np.int16)
    return np.tile(w, (8, 1))


def _a_expand(a, cfg):
    A = np.zeros((cfg.H, cfg.C, cfg.H), np.float32)
    for h in range(cfg.H):
        A[h, :, h] = a[h]
    return A.reshape(cfg.HID, cfg.H)


def preprocess(cfg, x, edges_idx, W1, a_src1, a_dst1, b1, W2, a_src2, a_dst2, b2):
    import ml_dtypes

    bfd = ml_dtypes.bfloat16
    f8d = ml_dtypes.float8_e4m3

    src = np.asarray(edges_idx[0], np.int64)
    dst = np.asarray(edges_idx[1], np.int64)
    order = np.argsort(dst, kind="stable")
    src_s, dst_s = src[order], dst[order]
    blk = dst_s // P
    counts = np.bincount(blk, minlength=cfg.NT)
    starts = np.concatenate([[0], np.cumsum(counts)])
    # per-block-position tile count (max over cores, so the SPMD program is
    # identical on every core)
    Tb = [
        max(1, int(-(-max(counts[c * cfg.BPC + b] for c in range(cfg.NC)) // P)))
        for b in range(cfg.BPC)
    ]
    Tmax = max(Tb)
    off = np.concatenate([[0], np.cumsum(Tb)]).astype(int)
    TT = int(off[-1])

    GBASE, GSIZE, GOFF = _group_maps(cfg)

    isrc = np.zeros((cfg.NC, cfg.BPC, P, 8 * Tmax), np.int16)
    selt = np.zeros((cfg.NC, TT, P, P), f8d)   # [e, d] per tile
    seld = np.zeros((cfg.NC, TT, P, P), f8d)   # [d, e] per tile
    iota = np.arange(P)
    RPC = cfg.BPC * P
    for gb in range(cfg.NT):
        c, b = gb // cfg.BPC, gb % cfg.BPC
        T = Tb[b]
        EPB = T * P
        s0, s1 = starts[gb], starts[gb + 1]
        n = s1 - s0
        a_src = np.zeros(EPB, np.int64)
        a_loc = np.full(EPB, -1, np.int64)
        a_src[:n] = src_s[s0:s1]
        a_loc[:n] = dst_s[s0:s1] - gb * P
        # table rows are group-major (AllGather layout; see AGB): node n at
        # base[g] + core*size[g] + (blk - b0[g]) * P + n % P
        a_b = (a_src % RPC) // P
        a_c = a_src // RPC
        a_row = GBASE[a_b] + a_c * GSIZE[a_b] + GOFF[a_b] * P + a_src % P
        isrc[c, b, :, : 8 * T] = _wrap16(a_row, T)
        loc_t = a_loc.reshape(T, P)                      # [t, e]
        st = (loc_t[:, :, None] == iota[None, None, :])  # [t, e, d]
        selt[c, off[b] : off[b] + T] = st.astype(f8d)
        seld[c, off[b] : off[b] + T] = np.swapaxes(st, 1, 2).astype(f8d)

    W1e = np.concatenate(
        [W1, W1 @ _a_expand(a_src1, cfg), W1 @ _a_expand(a_dst1, cfg)], axis=1
    ).astype(np.float32)
    W2e = np.concatenate(
        [W2, W2 @ _a_expand(a_src2, cfg), W2 @ _a_expand(a_dst2, cfg)], axis=1
    ).astype(np.float32)

    xT = np.zeros((cfg.HID, cfg.NP), np.float32)
    xT[:, : cfg.N] = np.asarray(x, np.float32).T
    b1b = np.broadcast_to(np.asarray(b1, np.float32), (P, cfg.HID)).copy()
    b2b = np.broadcast_to(np.asarray(b2, np.float32), (P, cfg.HID)).copy()

    shared = {
        "xT": xT.astype(bfd), "w1e": W1e.astype(bfd), "w2e": W2e.astype(bfd),
        "b1b": b1b, "b2b": b2b,
    }
    in_maps = [
        dict(
            shared,
            xtc=np.ascontiguousarray(xT[:, c * RPC : (c + 1) * RPC]).astype(bfd),
            isrc=isrc[c], selt=selt[c], seld=seld[c],
        )
        for c in range(cfg.NC)
    ]
    return in_maps, Tb


# --------------------------------------------------------------------------
# Device program
# --------------------------------------------------------------------------
def _ag_group(nc, cfg, stg, tab, b0, b1):
    """AllGather dst blocks [b0, b1) of per-core table rows into the
    group-major table (group base b0*NC*P, then rank-major)."""
    size = (b1 - b0) * P
    nc.gpsimd.collective_compute(
        "AllGather", OP.bypass,
        replica_groups=[list(range(cfg.NC))],
        ins=[stg[b0 * P : b1 * P, :]],
        outs=[tab[b0 * cfg.NC * P : b0 * cfg.NC * P + cfg.NC * size, :]],
    )


def _fat_from_psum(nc, cfg, fp, tps, dst_ap):
    """PSUM row block [h|s|d] -> fat tile [h|exp(s)|exp(.2s)|0] -> DMA."""
    HID = cfg.HID
    fat = fp.tile([P, cfg.FAT], BF16, tag="fat")
    nc.gpsimd.memset(fat[:, cfg.PQ1 : cfg.FAT], 0.0)
    nc.vector.tensor_copy(fat[:, 0:HID], tps[:, 0:HID])
    nc.scalar.activation(
        fat[:, cfg.PQ0 : cfg.PQ0 + 16].bitcast(F32), tps[:, HID : HID + 8], AF.Exp
    )
    nc.scalar.activation(
        fat[:, cfg.PQ0 + 16 : cfg.PQ1].bitcast(F32), tps[:, HID : HID + 8],
        AF.Exp, scale=0.2,
    )
    nc.scalar.dma_start(dst_ap, fat[:])


def _edge_phase(nc, tc, cfg, Tb, off, tab, tl, isrc_e, selt_e, seld_e,
                bias_t, layer, qn, *, l2=None, out_e=None):
    """l2 (layer 1 only): dict with w2a, w2b, idn, tl2, stg2, tabf2."""
    HID, H, NW2, FAT, BPC = cfg.HID, cfg.H, cfg.NW2, cfg.FAT, cfg.BPC
    Tmax = max(Tb)
    GG = 4  # tiles per work group (batches the small DVE ops)
    with (
        tc.tile_pool(name=f"ge{layer}", bufs=3) as gp,
        tc.tile_pool(name=f"ix{layer}", bufs=2) as ip,
        tc.tile_pool(name=f"sl{layer}", bufs=2) as slp,
        tc.tile_pool(name=f"wk{layer}", bufs=4) as wp,
        tc.tile_pool(name=f"eo{layer}", bufs=2) as op_,
        tc.tile_pool(name=f"eps{layer}", bufs=2, space="PSUM") as pp,
        tc.tile_pool(name=f"dps{layer}", bufs=2, space="PSUM") as dpp,
        tc.tile_pool(name=f"tps{layer}", bufs=1, space="PSUM") as tpp,
        tc.tile_pool(name=f"l2f{layer}", bufs=3) as l2p,
    ):
        for b in range(BPC):
            T = Tb[b]
            o = int(off[b])
            ist = ip.tile([P, 8 * Tmax], I16, tag="ist")
            nc.scalar.dma_start(ist[:, 0 : 8 * T], isrc_e[b, :, 0 : 8 * T])
            slt = slp.tile([P, Tmax, P], FP8, tag="slt")
            nc.sync.dma_start(
                slt[:, 0:T, :], selt_e[o : o + T].rearrange("t e d -> e t d")
            )
            sld = slp.tile([P, Tmax, P], FP8, tag="sld")
            nc.scalar.dma_start(
                sld[:, 0:T, :], seld_e[o : o + T].rearrange("t d e -> d t e")
            )
            gA = gp.tile([P, Tmax, FAT], BF16, tag="gA")
            # balanced gather chunks, each <= 8 tiles (SWDGE ring limit)
            nch = -(-T // 8)
            CH = -(-T // nch)
            for c0 in range(0, T, CH):
                cw = min(CH, T - c0)
                nc.gpsimd.dma_gather(
                    out_ap=gA[:, c0 : c0 + cw, :], in_ap=tab[:, :],
                    idxs_ap=ist[:, c0 * 8 : (c0 + cw) * 8],
                    num_idxs=P * cw, num_idxs_reg=P * cw, elem_size=FAT,
                    queue_num=qn[0] % 4,
                )
                qn[0] += 1

            ps = pp.tile([P, NW2], F32, tag="eps")
            for t0 in range(0, T, GG):
                tw = min(GG, T - t0)
                dx = dpp.tile([P, GG, 16], F32, tag="dx")
                for j in range(tw):
                    nc.tensor.matmul(
                        dx[:, j, :], sld[:, t0 + j, :],
                        tl[:, b * 16 : (b + 1) * 16],
                        start=True, stop=True,
                    )
                exf = wp.tile([P, GG, 16], F32, tag="exf")
                nc.vector.tensor_tensor(
                    exf[:, 0:tw, :], dx[:, 0:tw, :],
                    gA[:, t0 : t0 + tw, cfg.PQ0 : cfg.PQ1].bitcast(F32),
                    op=OP.mult,
                )
                rhs2 = wp.tile([P, GG, NW2], BF16, tag="rhs")
                nc.vector.tensor_tensor(
                    rhs2[:, 0:tw, HID : HID + H], exf[:, 0:tw, 0:8],
                    exf[:, 0:tw, 8:16], op=OP.max,
                )
                for j in range(tw):
                    t = t0 + j
                    nc.vector.tensor_tensor(
                        rhs2[:, j, 0:HID].rearrange("p (h c) -> p h c", h=H),
                        gA[:, t, 0:HID].rearrange("p (h c) -> p h c", h=H),
                        rhs2[:, j, HID : HID + H].to_broadcast([P, H, cfg.C]),
                        op=OP.mult,
                    )
                    nc.tensor.matmul(
                        ps[:], slt[:, t, :], rhs2[:, j, :],
                        start=(t == 0), stop=(t == T - 1),
                    )
            # epilogue
            den = wp.tile([P, H], F32, tag="den")
            nc.vector.tensor_scalar_add(den[:], ps[:, HID : HID + H], 1e-16)
            rec = wp.tile([P, H], F32, tag="rec")
            nc.vector.reciprocal(rec[:], den[:])
            ot = op_.tile([P, HID], F32, tag="ot")
            nc.vector.tensor_tensor(
                ot[:].rearrange("p (h c) -> p h c", h=H),
                ps[:, 0:HID].rearrange("p (h c) -> p h c", h=H),
                rec[:].to_broadcast([P, H, cfg.C]),
                op=OP.mult,
            )
            nc.vector.tensor_tensor(ot[:], ot[:], bias_t[:], op=OP.add)
            if layer == 1 and l2 is not None:
                # ELU(x) = relu(x) + exp(min(x,0)) - 1 -> bf16 z
                r_ = op_.tile([P, HID], F32, tag="relu")
                nc.scalar.activation(r_[:], ot[:], AF.Relu)
                m_ = op_.tile([P, HID], F32, tag="mneg")
                nc.vector.tensor_tensor(m_[:], ot[:], r_[:], op=OP.subtract)
                nc.scalar.activation(m_[:], m_[:], AF.Exp)
                nc.vector.tensor_scalar_add(m_[:], m_[:], -1.0)
                zt = op_.tile([P, HID], BF16, tag="zt")
                nc.vector.tensor_tensor(zt[:], r_[:], m_[:], op=OP.add)
                if out_e is not None:
                    nc.sync.dma_start(out_e[b * P : (b + 1) * P, :], ot[:])
                # layer-2 table rows for this block: transpose z on the PE,
                # multiply by W2_ext, exp the attention slots, AllGather.
                ptA = tpp.tile([P, P], F32, tag="ptA")
                nc.tensor.matmul(ptA[:], zt[:, 0:P], l2["idn"][:],
                                 start=True, stop=True)
                ptB = tpp.tile([P, P], F32, tag="ptB")
                nc.tensor.matmul(ptB[:], zt[:, P : 2 * P], l2["idn"][:],
                                 start=True, stop=True)
                zT0 = op_.tile([P, P], BF16, tag="zT0")
                nc.vector.tensor_copy(zT0[:], ptA[:])
                zT1 = op_.tile([P, P], BF16, tag="zT1")
                nc.vector.tensor_copy(zT1[:], ptB[:])
                tp2 = tpp.tile([P, cfg.NW], F32, tag="tps2")
                nc.tensor.matmul(tp2[:], zT0[:], l2["w2a"][:],
                                 start=True, stop=False)
                nc.tensor.matmul(tp2[:], zT1[:], l2["w2b"][:],
                                 start=False, stop=True)
                stg2 = l2["stg2"]
                _fat_from_psum(nc, cfg, l2p, tp2, stg2[b * P : (b + 1) * P, :])
                tl2 = l2["tl2"]
                nc.scalar.activation(tl2[:, b * 16 : b * 16 + 8],
                                     tp2[:, HID + 8 : HID + 16], AF.Exp)
                nc.scalar.activation(tl2[:, b * 16 + 8 : b * 16 + 16],
                                     tp2[:, HID + 8 : HID + 16], AF.Exp, scale=0.2)
                for b0, b1 in AGB:
                    if b1 - 1 == b:
                        _ag_group(nc, cfg, stg2, l2["tabf2"], b0, b1)
            else:
                nc.sync.dma_start(out_e[b * P : (b + 1) * P, :], ot[:])


def build_program(cfg, Tb, stages="full"):
    GBASE, GSIZE, GOFF = _group_maps(cfg)
    nc = bacc.Bacc(num_swdge_queues=4)
    HID, NW, BPC, NC, NT = cfg.HID, cfg.NW, cfg.BPC, cfg.NC, cfg.NT
    Tmax = max(Tb)
    off = np.concatenate([[0], np.cumsum(Tb)]).astype(int)
    TT = int(off[-1])
    RPC = BPC * P

    xT_e = nc.declare_dram_parameter("xT", [HID, cfg.NP], BF16, isOutput=False)
    xtc_e = nc.declare_dram_parameter("xtc", [HID, RPC], BF16, isOutput=False)
    w1_e = nc.declare_dram_parameter("w1e", [HID, NW], BF16, isOutput=False)
    w2_e = nc.declare_dram_parameter("w2e", [HID, NW], BF16, isOutput=False)
    b1_e = nc.declare_dram_parameter("b1b", [P, HID], F32, isOutput=False)
    b2_e = nc.declare_dram_parameter("b2b", [P, HID], F32, isOutput=False)
    isrc_e = nc.declare_dram_parameter("isrc", [BPC, P, 8 * Tmax], I16, isOutput=False)
    selt_e = nc.declare_dram_parameter("selt", [TT, P, P], FP8, isOutput=False)
    seld_e = nc.declare_dram_parameter("seld", [TT, P, P], FP8, isOutput=False)
    out_e = nc.declare_dram_parameter("out", [RPC, HID], F32, isOutput=True)

    stg2 = nc.dram_tensor("stg2", [RPC, cfg.FAT], BF16)
    tabf1 = nc.dram_tensor("tabf1", [cfg.NP, cfg.FAT], BF16)
    tabf2 = nc.dram_tensor("tabf2", [cfg.NP, cfg.FAT], BF16, addr_space="Shared")

    qn = [0]
    with tile.TileContext(nc) as tc:
        with tc.tile_pool(name="const", bufs=1) as cp:
            w1a = cp.tile([P, NW], BF16)
            nc.sync.dma_start(w1a[:], w1_e[0:P, :])
            w1b = cp.tile([P, NW], BF16)
            nc.sync.dma_start(w1b[:], w1_e[P : 2 * P, :])
            w2a = cp.tile([P, NW], BF16)
            nc.sync.dma_start(w2a[:], w2_e[0:P, :])
            w2b = cp.tile([P, NW], BF16)
            nc.sync.dma_start(w2b[:], w2_e[P : 2 * P, :])
            b1t = cp.tile([P, HID], F32)
            nc.sync.dma_start(b1t[:], b1_e[:, :])
            b2t = cp.tile([P, HID], F32)
            nc.sync.dma_start(b2t[:], b2_e[:, :])
            idn = cp.tile([P, P], BF16)
            make_identity(nc, idn[:])
            tl1 = cp.tile([P, BPC * 16], BF16)
            tl2 = cp.tile([P, BPC * 16], BF16)

            # ---- dst-side tails for layer 1 (own nodes only, from xtc) ----
            with (
                tc.tile_pool(name="xtl", bufs=1) as xtl,
                tc.tile_pool(name="pst", bufs=2, space="PSUM") as pst,
            ):
                xq0 = xtl.tile([P, RPC], BF16)
                nc.sync.dma_start(xq0[:], xtc_e[0:P, :])
                xq1 = xtl.tile([P, RPC], BF16)
                nc.sync.dma_start(xq1[:], xtc_e[P : 2 * P, :])
                for b in range(BPC):
                    pd = pst.tile([P, 8], F32, tag="pd")
                    nc.tensor.matmul(pd[:], xq0[:, b * P : (b + 1) * P],
                                     w1a[:, HID + 8 : HID + 16],
                                     start=True, stop=False)
                    nc.tensor.matmul(pd[:], xq1[:, b * P : (b + 1) * P],
                                     w1b[:, HID + 8 : HID + 16],
                                     start=False, stop=True)
                    nc.scalar.activation(tl1[:, b * 16 : b * 16 + 8], pd[:], AF.Exp)
                    nc.scalar.activation(tl1[:, b * 16 + 8 : b * 16 + 16], pd[:],
                                         AF.Exp, scale=0.2)

            # ---- layer-1 table (full, redundant per core, block-major) ----
            # Panels of 10 node blocks: per-block matmuls into PSUM, h copied
            # into a panel-wide fat tile, exps batched per panel (2 scalar
            # ops), then one table write per block on the sync queue.
            PAN = 10
            with (
                tc.tile_pool(name="s1", bufs=2) as sp,
                tc.tile_pool(name="fp1", bufs=2) as fp,
                tc.tile_pool(name="ps1", bufs=3, space="PSUM") as pp,
            ):
                xf0 = sp.tile([P, NT * P], BF16, tag="xf0")
                nc.sync.dma_start(xf0[:], xT_e[0:P, :])
                xf1 = sp.tile([P, NT * P], BF16, tag="xf1")
                nc.scalar.dma_start(xf1[:], xT_e[P : 2 * P, :])
                for pan in range(-(-NT // PAN)):
                    g0, g1 = pan * PAN, min(NT, (pan + 1) * PAN)
                    gw = g1 - g0
                    # node-panel pan covers core pan's blocks 0..9 (gb%BPC)
                    fatp = fp.tile([P, PAN, cfg.FAT], BF16, tag="fatp")
                    nc.gpsimd.memset(fatp[:, :, cfg.PQ1 : cfg.FAT], 0.0)
                    sdp = sp.tile([P, PAN, 8], F32, tag="sdp")
                    for gb in range(g0, g1):
                        i = gb - g0
                        oo = gb * P
                        tps = pp.tile([P, NW], F32, tag="tps")
                        nc.tensor.matmul(tps[:], xf0[:, oo : oo + P], w1a[:],
                                         start=True, stop=False)
                        nc.tensor.matmul(tps[:], xf1[:, oo : oo + P], w1b[:],
                                         start=False, stop=True)
                        if i % 2 == 0:
                            nc.scalar.activation(fatp[:, i, 0:HID],
                                                 tps[:, 0:HID], AF.Copy)
                        else:
                            nc.vector.tensor_copy(fatp[:, i, 0:HID],
                                                  tps[:, 0:HID])
                        nc.vector.tensor_copy(sdp[:, i, :], tps[:, HID : HID + 8])
                    nc.scalar.activation(
                        fatp[:, 0:gw, cfg.PQ0 : cfg.PQ0 + 16].bitcast(F32),
                        sdp[:, 0:gw, :], AF.Exp,
                    )
                    nc.scalar.activation(
                        fatp[:, 0:gw, cfg.PQ0 + 16 : cfg.PQ1].bitcast(F32),
                        sdp[:, 0:gw, :], AF.Exp, scale=0.2,
                    )
                    for gi, (b0, b1) in enumerate(AGB):
                        # whole AG group is contiguous in the fat panel and
                        # in the table: one write per group
                        r0 = int(GBASE[b0] + pan * GSIZE[b0])
                        sz = (b1 - b0) * P
                        eng = nc.sync if gi % 2 == 0 else nc.scalar
                        eng.dma_start(
                            tabf1[r0 : r0 + sz, :].rearrange(
                                "(j p) f -> p j f", p=P),
                            fatp[:, b0:b1, :])

            if stages == "z1":
                l2 = dict(w2a=w2a, w2b=w2b, idn=idn, tl2=tl2, stg2=stg2, tabf2=tabf2)
                _edge_phase(nc, tc, cfg, Tb, off, tabf1, tl1, isrc_e, selt_e, seld_e,
                            b1t, layer=1, qn=qn, l2=l2, out_e=out_e)
            elif stages == "z1nol2":
                _edge_phase(nc, tc, cfg, Tb, off, tabf1, tl1, isrc_e, selt_e, seld_e,
                            b1t, layer=1, qn=qn, l2=None, out_e=out_e)
            elif stages == "stg2":
                l2 = dict(w2a=w2a, w2b=w2b, idn=idn, tl2=tl2, stg2=stg2, tabf2=tabf2)
                _edge_phase(nc, tc, cfg, Tb, off, tabf1, tl1, isrc_e, selt_e, seld_e,
                            b1t, layer=1, qn=qn, l2=l2)
                with tc.tile_pool(name="dbg", bufs=2) as dp:
                    for b in range(BPC):
                        dt_ = dp.tile([P, cfg.FAT], BF16, tag="dbg")
                        nc.sync.dma_start(dt_[:], stg2[b * P : (b + 1) * P, :])
                        dt2 = dp.tile([P, HID], F32, tag="dbg2")
                        nc.vector.tensor_copy(dt2[:], dt_[:, 0:HID])
                        nc.sync.dma_start(out_e[b * P : (b + 1) * P, :], dt2[:])
            elif stages == "tab2":
                l2 = dict(w2a=w2a, w2b=w2b, idn=idn, tl2=tl2, stg2=stg2, tabf2=tabf2)
                _edge_phase(nc, tc, cfg, Tb, off, tabf1, tl1, isrc_e, selt_e, seld_e,
                            b1t, layer=1, qn=qn, l2=l2)
                with tc.tile_pool(name="dbg", bufs=2) as dp:
                    for b in range(BPC):
                        dt_ = dp.tile([P, cfg.FAT], BF16, tag="dbg")
                        nc.sync.dma_start(dt_[:], tabf2[b * P : (b + 1) * P, :])
                        dt2 = dp.tile([P, HID], F32, tag="dbg2")
                        nc.vector.tensor_copy(dt2[:], dt_[:, 0:HID])
                        nc.sync.dma_start(out_e[b * P : (b + 1) * P, :], dt2[:])
            elif stages == "tab1":
                with tc.tile_pool(name="dbg", bufs=2) as dp:
                    for b in range(BPC):
                        dt_ = dp.tile([P, cfg.FAT], BF16, tag="dbg")
                        nc.sync.dma_start(dt_[:], tabf1[b * P : (b + 1) * P, :])
                        dt2 = dp.tile([P, HID], F32, tag="dbg2")
                        nc.vector.tensor_copy(dt2[:], dt_[:, 0:HID])
                        nc.sync.dma_start(out_e[b * P : (b + 1) * P, :], dt2[:])
            elif stages == "tl1":
                with tc.tile_pool(name="dbg", bufs=2) as dp:
                    for b in range(BPC):
                        dt2 = dp.tile([P, HID], F32, tag="dbg2")
                        nc.gpsimd.memset(dt2[:], 0.0)
                        nc.vector.tensor_copy(dt2[:, 0:16], tl1[:, b * 16 : (b + 1) * 16])
                        nc.sync.dma_start(out_e[b * P : (b + 1) * P, :], dt2[:])
            else:
                l2 = dict(w2a=w2a, w2b=w2b, idn=idn, tl2=tl2, stg2=stg2, tabf2=tabf2)
                _edge_phase(nc, tc, cfg, Tb, off, tabf1, tl1, isrc_e, selt_e, seld_e,
                            b1t, layer=1, qn=qn, l2=l2)
                _edge_phase(nc, tc, cfg, Tb, off, tabf2, tl2, isrc_e, selt_e, seld_e,
                            b2t, layer=2, qn=qn, out_e=out_e)
    nc.finalize()
    return nc


# --------------------------------------------------------------------------
# Entry point
# --------------------------------------------------------------------------
def run_gat(inputs, cfg=None, trace=False, stages="full"):
    cfg = cfg or Cfg()
    in_maps, Tb = preprocess(cfg, **inputs)
    nc = build_program(cfg, Tb, stages=stages)
    res = run_bass_kernel_spmd(nc, in_maps, list(range(cfg.NC)), trace=trace)
    out = np.concatenate([res.results[c]["out"] for c in range(cfg.NC)], axis=0)
    return out[: cfg.N], res


def kernel(**inputs) -> np.ndarray:
    out, _ = run_gat(inputs)
    return np.ascontiguousarray(out, dtype=np.float32)
